# revision 66
# baseline (speedup 1.0000x reference)
"""Sliding-window causal self-attention (T=4096, D=1024, 16 heads, window=1024)
on 8 Trainium2 NeuronCores.

Sharding: tensor-parallel over heads — 2 heads per core. Each core computes
qkv projection for its heads (sliced w_qkv rows), full attention for its
heads, and a partial output projection (sliced w_o columns, 0.5*v folded in).
The 8 partial [T, D] outputs are summed on the host (the all-reduce).

All matmul operands are fp16 (10-bit mantissa, 1 PE cycle/row at any
moving-dim size vs 4 for fp32, 2-byte DMA/SBUF footprint); PSUM
accumulation and the softmax denominator / rmsnorm math stay fp32.

Schedule: a single software-pipelined loop over 16 query-block pairs.
Iteration p leads with the attention (scores -> exp -> masks -> p@v) of
pair p, whose q/k/v were produced two iterations earlier, then fills the
engine queues with the qkv projection + rmsnorm + rope for pair p+2 and
the w_o output projection of pair p-1. rsqrt is computed as
Exp(-0.5*Ln(x)) so the Activation engine never swaps its function table
between the rmsnorm and the attention exp (a 1.3us reload each time).
"""

from contextlib import ExitStack

import numpy as np

import concourse.mybir as mybir
import concourse.tile as tile
from concourse import bacc
from concourse.bass import ts
from concourse.bass_utils import run_bass_kernel_spmd
from concourse.masks import make_identity

F32 = mybir.dt.float32
F16 = mybir.dt.float16

T = 4096
C = 1024
H = 16
DH = 64
N_CORES = 8
HPC = H // N_CORES  # heads per core = 2
NB = T // 128       # 32 token blocks
NP = NB // 2        # 16 query-block pairs
WINDOW = 1024
LOOKBACK = WINDOW // 128
SCALE = 0.12
EPS = 1e-6
ROT = DH // 2   # 32 rotary channels


def _pin_act_set(arch):
    """Make every activation func this kernel uses (Ln, Exp, Copy) resolve
    to the one table set that contains them all. insert_act_table_loads
    picks the FIRST set containing a func, so Exp->exp_and_others and
    Ln->natural_log by default, forcing a 1.3us table reload at every
    Ln/Exp alternation. Stripping the shared funcs from the earlier sets
    (in the cached dict, indices unchanged) routes all of them to
    natural_log_exp_and_others -> a single load for the whole kernel."""
    from concourse.hw_specs import get_activation_tables
    tabs = get_activation_tables(arch)
    target = "natural_log_exp_and_others"
    keep = tabs[target]
    for name, s in tabs.items():
        if name == target:
            break
        s -= keep


def build_program():
    nc = bacc.Bacc("TRN2", target_bir_lowering=False, debug=False,
                   num_devices=N_CORES)
    _pin_act_set(nc.m.arch)

    xT = nc.dram_tensor("xT", [C, T], F16, kind="ExternalInput").ap()
    wT = nc.dram_tensor("wT", [C, 6 * DH], F16, kind="ExternalInput").ap()
    w_oT = nc.dram_tensor("w_oT", [2 * DH, C], F16, kind="ExternalInput").ap()
    cs2 = nc.dram_tensor("cs2", [T, 8 * ROT], F16, kind="ExternalInput").ap()
    nw = nc.dram_tensor("nw", [128, 256], F16, kind="ExternalInput").ap()
    # masks in scoresT layout [key j (partition), query i (free)]
    mf = nc.dram_tensor("mf", [128, 128], F16, kind="ExternalInput").ap()
    ml = nc.dram_tensor("ml", [128, 128], F16, kind="ExternalInput").ap()
    outp = nc.dram_tensor("outp", [T, C], F16, kind="ExternalOutput").ap()

    with tile.TileContext(nc) as tc:
        _body(tc, xT, wT, w_oT, cs2, nw, mf, ml, outp)
    nc.compile()
    return nc


def _body(tc, xT, wT, w_oT, cs2, nw, mf, ml, outp):
    nc = tc.nc
    with ExitStack() as octx:
        singles = octx.enter_context(tc.tile_pool(name="singles", bufs=1))

        # persistent SBUF residents, d-major: head h occupies partitions
        # [64h, 64h+64); score matmuls contract over just that head's 64
        # partitions, so no zero-padding or per-head slots are needed.
        qTr = singles.tile([128, T], F16)
        kTr = singles.tile([128, T], F16)
        # v + ones column, token-major: [tok%128, block, head, 64+1]
        v_all = singles.tile([128, NB, HPC, DH + 1], F16)
        wT_sb = singles.tile([128, 8, 6 * DH], F16)
        w_oT_sb = singles.tile([128, C], F16)
        nw_sb = singles.tile([128, 256], F16)
        mf_sb = singles.tile([128, 128], F16)
        ml_sb = singles.tile([128, 128], F16)
        ident = singles.tile([128, 128], F16)
        eps_sb = singles.tile([128, 1], F32)

        nc.sync.dma_start(wT_sb, wT.rearrange("(a p) n -> p a n", p=128))
        nc.sync.dma_start(w_oT_sb, w_oT)
        nc.sync.dma_start(nw_sb, nw)
        nc.sync.dma_start(mf_sb, mf)
        nc.sync.dma_start(ml_sb, ml)
        make_identity(nc, ident)
        nc.vector.memset(eps_sb, EPS)
        nc.vector.memset(v_all[:, :, :, DH:DH + 1], 1.0)

        xT_r = xT.rearrange("(a p) t -> p a t", p=128)

        # PSUM budget (8 banks, tiles are bank-rounded): pq 2, sc 2, av 1,
        # tp 1, wo 2. The four p@v accumulators of a pair share one av
        # tile; the six transpose slots of an iteration share one tp tile.
        psum_pq = octx.enter_context(
            tc.tile_pool(name="psum_pq", bufs=2, space="PSUM"))
        psum_sc = octx.enter_context(
            tc.tile_pool(name="psum_sc", bufs=2, space="PSUM"))
        psum_tp = octx.enter_context(
            tc.tile_pool(name="psum_tp", bufs=1, space="PSUM"))
        psum_av = octx.enter_context(
            tc.tile_pool(name="psum_av", bufs=1, space="PSUM"))
        psum_wo = octx.enter_context(
            tc.tile_pool(name="psum_wo", bufs=1, space="PSUM"))
        awork = octx.enter_context(tc.tile_pool(name="awork", bufs=5))
        bwork = octx.enter_context(tc.tile_pool(name="bwork", bufs=3))
        bsmall = octx.enter_context(tc.tile_pool(name="bsmall", bufs=4))

        # -------- per-token-block: qkv projection + rmsnorm + rope ---------
        def dma_block(i):
            xt = awork.tile([128, 8, 128], F16, tag="xt", name="xt")
            nc.sync.dma_start(xt, xT_r[:, :, ts(i, 128)])
            csn = awork.tile([128, 2, 4, ROT], F16, tag="csn", name="csn")
            nc.sync.dma_start(csn, cs2[ts(i, 128), :].rearrange(
                "p (t g r) -> p t g r", t=2, g=4))
            return xt, csn[:, 0], csn[:, 1]

        def do_block(i, tiles):
            xt, cs, sn = tiles
            pq = psum_pq.tile([128, 6 * DH], F32, tag="proj", name="pq")
            for j in range(8):
                nc.tensor.matmul(pq, lhsT=xt[:, j, :], rhs=wT_sb[:, j, :],
                                 start=(j == 0), stop=(j == 7))

            # rmsnorm over each of the 4 q/k head groups (64 ch each).
            # HW allows only one non-scalar PSUM operand per instruction, so
            # stage q/k into SBUF fp16 first (also unlocks DVE 2-byte modes).
            qk_sb = awork.tile([128, 256], F16, tag="qk_sb", name="qk_sb")
            nc.vector.tensor_copy(qk_sb, pq[:, 0:256])
            pq4 = qk_sb.rearrange("p (g d) -> p g d", g=4)
            sq = awork.tile([128, 4, DH], F16, tag="sq", name="sq")
            nc.vector.tensor_mul(sq, pq4, pq4)
            ms = awork.tile([128, 4], F32, tag="ms", name="ms")
            nc.vector.reduce_sum(ms, sq, axis=mybir.AxisListType.X)
            # rinv = (ms/DH + eps)^-0.5 via Ln+Exp: both live in the same
            # activation-function set as the attention Exp, so interleaving
            # them never triggers a 1.3us activation-table reload.
            lg = awork.tile([128, 4], F32, tag="lg", name="lg")
            nc.scalar.activation(lg, ms, mybir.ActivationFunctionType.Ln,
                                 bias=eps_sb, scale=1.0 / DH)
            rinv = awork.tile([128, 4], F32, tag="rinv", name="rinv")
            nc.scalar.activation(rinv, lg, mybir.ActivationFunctionType.Exp,
                                 scale=-0.5)

            qkn = awork.tile([128, 256], F16, tag="qkn", name="qkn")
            qkn4 = qkn.rearrange("p (g d) -> p g d", g=4)
            nw4 = nw_sb.rearrange("p (g d) -> p g d", g=4)
            for g in range(4):
                nc.vector.scalar_tensor_tensor(
                    qkn4[:, g, :], pq4[:, g, :], rinv[:, g:g + 1],
                    nw4[:, g, :], mybir.AluOpType.mult,
                    mybir.AluOpType.mult)

            # half-rope on first 32 channels of each group:
            # q' = q*C + rot(q)*S, rot(q) = interleave(-odd, even)
            r4v = qkn4[:, :, 0:ROT].rearrange("p g (a two) -> p g a two",
                                              two=2)
            even = r4v[:, :, :, 0]
            odd = r4v[:, :, :, 1]
            rot = awork.tile([128, 4, ROT], F16, tag="rot", name="rot")
            rot2 = rot.rearrange("p g (a two) -> p g a two", two=2)
            nc.gpsimd.tensor_scalar_mul(rot2[:, :, :, 0], odd, -1.0)
            nc.gpsimd.tensor_copy(rot2[:, :, :, 1], even)
            nc.gpsimd.tensor_mul(rot, rot, sn)
            f16v = qkn4[:, :, 0:ROT]
            nc.gpsimd.tensor_mul(f16v, f16v, cs)
            nc.gpsimd.tensor_add(f16v, f16v, rot)

            # GPSIMD cannot read PSUM on HW; the Act engine stages v out
            nc.scalar.copy(
                v_all[:, i, :, 0:DH],
                pq[:, 256:384].rearrange("p (h d) -> p h d", h=HPC))
            return qkn

        # transposes are emitted separately, AFTER attention matmuls, so
        # the PE queue never head-of-line blocks on the pool rope chain
        def do_block_tp(i, qkn, tp6, sl):
            tpq = tp6[:, sl, :]
            nc.tensor.transpose(tpq, qkn[:, 0:128], ident)
            nc.vector.tensor_copy(qTr[:, ts(i, 128)], tpq)
            tpk = tp6[:, sl + 1, :]
            nc.tensor.transpose(tpk, qkn[:, 128:256], ident)
            nc.vector.tensor_copy(kTr[:, ts(i, 128)], tpk)

        # -------- per 256-query pair: attention + output projection --------
        attn_tiles = {}

        def emit_unit(p, h, av4):
            nkp = min(2 * p, LOOKBACK) + 2
            k0 = 2 * p + 1 - (nkp - 1)
            hp = 64 * h
            pT = bwork.tile([128, nkp, 256], F16, tag="pT", name="pT")
            # scoresT chunks (2 per PSUM bank) + exp. The masked chunk
            # pairs are computed FIRST so their pool-engine mask multiplies
            # overlap the remaining exps instead of delaying p@v.
            pairs = [(j, min(2, nkp - j)) for j in range(0, nkp, 2)]
            pairs = pairs[-1:] + pairs[:-1]
            for j, w in pairs:
                sc = psum_sc.tile([128, w, 256], F32, tag="sc", name="sc")
                for jj in range(w):
                    nc.tensor.matmul(
                        sc[:, jj, :],
                        lhsT=kTr[64 * h:64 * h + 64, ts(k0 + j + jj, 128)],
                        rhs=qTr[64 * h:64 * h + 64, ts(p, 256)],
                        start=True, stop=True)
                nc.scalar.activation(pT[:, j:j + w, :], sc,
                                     mybir.ActivationFunctionType.Exp,
                                     scale=SCALE)
                # window-edge + causal masks (multiplicative, post-exp),
                # issued as soon as their chunk's exp is available
                if j <= 0 < j + w and p >= 4:
                    nc.gpsimd.tensor_mul(pT[:, 0, 0:128], pT[:, 0, 0:128],
                                         mf_sb)
                if j <= 1 < j + w and p >= 4:
                    nc.gpsimd.tensor_mul(pT[:, 1, 128:256],
                                         pT[:, 1, 128:256], mf_sb)
                if j <= nkp - 2 < j + w:
                    nc.gpsimd.tensor_mul(pT[:, nkp - 2, 0:128],
                                         pT[:, nkp - 2, 0:128], ml_sb)
                if j <= nkp - 1 < j + w:
                    nc.gpsimd.tensor_mul(pT[:, nkp - 1, 128:256],
                                         pT[:, nkp - 1, 128:256], ml_sb)
            # p @ [v | 1] per 128-query half
            for q in (0, 1):
                blk = 2 * p + q
                lo = 1 if (q == 1 and p >= 4) else 0
                hi = nkp - 2 if q == 0 else nkp - 1
                av = av4[:, 2 * h + q, :]
                js = list(range(lo, hi + 1))
                for m, jx in enumerate(js):
                    nc.tensor.matmul(
                        av, lhsT=pT[:, jx, 128 * q:128 * q + 128],
                        rhs=v_all[:, k0 + jx, h, :],
                        start=(m == 0), stop=(m == len(js) - 1))

        def emit_tail(blk, tp):
            at = attn_tiles.pop(blk)
            nc.tensor.transpose(tp, at, ident)
            aT = bsmall.tile([128, 128], F16, tag="aT", name="aT")
            nc.vector.tensor_copy(aT, tp)
            wo_ps = psum_wo.tile([128, C], F32, tag="wo", name="wo_ps")
            nc.tensor.matmul(wo_ps[:, 0:512], lhsT=aT,
                             rhs=w_oT_sb[:, 0:512], start=True, stop=True)
            nc.tensor.matmul(wo_ps[:, 512:1024], lhsT=aT,
                             rhs=w_oT_sb[:, 512:1024], start=True,
                             stop=True)
            wo_sb = bwork.tile([128, C], F16, tag="wo_sb", name="wo_sb")
            nc.vector.tensor_copy(wo_sb, wo_ps)
            nc.sync.dma_start(outp[ts(blk, 128), :], wo_sb)

        # -------- interleaved schedule: qkv for pair p, then attention -----
        # Software-pipelined, attention-first: each iteration leads with the
        # attention of pair p (inputs ready since last iteration), then the
        # qkv/rope for pair p+1 fills the engines' tails. x/cos/sin DMAs are
        # issued a full iteration ahead of their compute.
        pending = []
        tp0 = psum_tp.tile([128, 6, 128], F16, tag="tp", name="tp0")
        tp1 = psum_tp.tile([128, 6, 128], F16, tag="tp", name="tp1")
        t0, t1 = dma_block(0), dma_block(1)
        t2, t3 = dma_block(2), dma_block(3)
        do_block_tp(0, do_block(0, t0), tp0, 0)
        do_block_tp(1, do_block(1, t1), tp0, 2)
        do_block_tp(2, do_block(2, t2), tp1, 0)
        do_block_tp(3, do_block(3, t3), tp1, 2)
        nxt = (dma_block(4), dma_block(5)) if NP > 2 else None
        for p in range(NP):
            tp6 = psum_tp.tile([128, 6, 128], F16, tag="tp", name="tp6")
            if p + 3 < NP:
                cur, nxt = nxt, (dma_block(2 * p + 6), dma_block(2 * p + 7))
            else:
                cur, nxt = nxt, None
            attn_tiles[2 * p] = bsmall.tile([128, 128], F16,
                                            tag="attn", name="attn_a")
            attn_tiles[2 * p + 1] = bsmall.tile([128, 128], F16,
                                                tag="attn", name="attn_b")
            av4 = psum_av.tile([128, 4, DH + 1], F32, tag="av", name="av4")
            emit_unit(p, 0, av4)
            if cur is not None:
                qa = do_block(2 * p + 4, cur[0])
            for n, blk in enumerate(pending):
                emit_tail(blk, tp6[:, 4 + n, :])
            pending = []
            emit_unit(p, 1, av4)
            if cur is not None:
                qb = do_block(2 * p + 5, cur[1])
                do_block_tp(2 * p + 4, qa, tp6, 0)
                do_block_tp(2 * p + 5, qb, tp6, 2)
            r4 = bsmall.tile([128, 4], F32, tag="r4", name="r4")
            r4s = bsmall.tile([128, 4], F32, tag="r4s", name="r4s")
            nc.vector.reciprocal_approx_accurate(r4, av4[:, :, DH], r4s)
            # normalize straight out of PSUM into the attn tiles: one TSP
            # replaces the copy + in-place scale
            for h in range(2):
                for q in range(2):
                    nc.vector.tensor_scalar_mul(
                        attn_tiles[2 * p + q][:, 64 * h:64 * h + 64],
                        av4[:, 2 * h + q, 0:DH],
                        r4[:, 2 * h + q:2 * h + q + 1])
            pending = [2 * p, 2 * p + 1]
        tp_last = psum_tp.tile([128, 6, 128], F16, tag="tp", name="tp_last")
        for n, blk in enumerate(pending):
            emit_tail(blk, tp_last[:, n, :])


_PROGRAM = None


def _get_program():
    global _PROGRAM
    if _PROGRAM is None:
        _PROGRAM = build_program()
    return _PROGRAM


def make_in_maps(x, w_qkv, w_o, cos, sin, pos):
    """Host-side sharding: build the per-core input dicts."""
    xT = np.ascontiguousarray(
        np.asarray(x, np.float32).reshape(T, C).T).astype(np.float16)

    cos_u = np.asarray(cos, np.float32)[np.asarray(pos).reshape(-1)]
    sin_u = np.asarray(sin, np.float32)[np.asarray(pos).reshape(-1)]
    cdup = np.empty((T, ROT), np.float32)
    cdup[:, 0::2] = cos_u
    cdup[:, 1::2] = cos_u
    sdup = np.empty((T, ROT), np.float32)
    sdup[:, 0::2] = sin_u
    sdup[:, 1::2] = sin_u
    cs2 = np.concatenate([np.tile(cdup, (1, 4)), np.tile(sdup, (1, 4))],
                         axis=1).astype(np.float16)

    ones = np.ones((128, 128), np.float16)
    mf = np.ascontiguousarray(np.tril(ones, -1))  # allowed iff q_i < key_j
    ml = np.ascontiguousarray(np.triu(ones, 0))   # allowed iff q_i >= key_j

    w_qkv = np.asarray(w_qkv, np.float32)
    w_o = np.asarray(w_o, np.float32)
    in_maps = []
    for c in range(N_CORES):
        h0, h1 = HPC * c, HPC * c + 1
        rows = np.r_[h0 * DH:(h0 + 1) * DH, h1 * DH:(h1 + 1) * DH]
        w_shard = np.concatenate(
            [w_qkv[rows], w_qkv[C + rows], w_qkv[2 * C + rows]], axis=0)
        wT_c = np.ascontiguousarray(w_shard.T).astype(np.float16)
        w_oT_c = np.ascontiguousarray((0.5 * w_o[:, rows]).T).astype(np.float16)
        in_maps.append({
            "xT": xT, "wT": wT_c, "w_oT": w_oT_c,
            "cs2": cs2, "nw": None, "mf": mf, "ml": ml,
        })
    return in_maps


def _norm_weight_tile(q_norm_w, k_norm_w):
    nw = np.concatenate([np.tile(np.asarray(q_norm_w, np.float32), HPC),
                         np.tile(np.asarray(k_norm_w, np.float32), HPC)])
    return np.ascontiguousarray(np.broadcast_to(nw, (128, 256))).astype(
        np.float16)


def kernel(x, tokens, pos, w_qkv, w_o, q_norm_w, k_norm_w, cos, sin,
           window_tokens, block_size):
    assert int(window_tokens) == WINDOW and int(block_size) == 128
    nc = _get_program()
    in_maps = make_in_maps(x, w_qkv, w_o, cos, sin, pos)
    nw_t = _norm_weight_tile(q_norm_w, k_norm_w)
    for m in in_maps:
        m["nw"] = nw_t

    res = run_bass_kernel_spmd(nc, in_maps, list(range(N_CORES)))
    out = np.zeros((T, C), np.float64)
    for c in range(N_CORES):
        out += res.results[c]["outp"].astype(np.float64)
    return out.astype(np.float32).reshape(1, T, C)



# revision 75
# speedup vs baseline: 1.0017x; 1.0017x over previous
"""Sliding-window causal self-attention (T=4096, D=1024, 16 heads, window=1024)
on 8 Trainium2 NeuronCores.

Sharding: tensor-parallel over heads — 2 heads per core. Each core computes
qkv projection for its heads (sliced w_qkv rows), full attention for its
heads, and a partial output projection (sliced w_o columns, 0.5*v folded in).
The 8 partial [T, D] outputs are summed on the host (the all-reduce).

All matmul operands are fp16 (10-bit mantissa, 1 PE cycle/row at any
moving-dim size vs 4 for fp32, 2-byte DMA/SBUF footprint); PSUM
accumulation and the softmax denominator / rmsnorm math stay fp32.

Schedule: a single software-pipelined loop over 16 query-block pairs.
Iteration p leads with the attention (scores -> exp -> masks -> p@v) of
pair p, whose q/k/v were produced two iterations earlier, then fills the
engine queues with the qkv projection + rmsnorm + rope for pair p+2 and
the w_o output projection of pair p-1. rsqrt is computed as
Exp(-0.5*Ln(x)) so the Activation engine never swaps its function table
between the rmsnorm and the attention exp (a 1.3us reload each time).
"""

from contextlib import ExitStack

import numpy as np

import concourse.mybir as mybir
import concourse.tile as tile
from concourse import bacc
from concourse.bass import ts
from concourse.bass_utils import run_bass_kernel_spmd
from concourse.masks import make_identity

F32 = mybir.dt.float32
F16 = mybir.dt.float16

T = 4096
C = 1024
H = 16
DH = 64
N_CORES = 8
HPC = H // N_CORES  # heads per core = 2
NB = T // 128       # 32 token blocks
NP = NB // 2        # 16 query-block pairs
WINDOW = 1024
LOOKBACK = WINDOW // 128
SCALE = 0.12
EPS = 1e-6
ROT = DH // 2   # 32 rotary channels


def _pin_act_set(arch):
    """Make every activation func this kernel uses (Ln, Exp, Copy) resolve
    to the one table set that contains them all. insert_act_table_loads
    picks the FIRST set containing a func, so Exp->exp_and_others and
    Ln->natural_log by default, forcing a 1.3us table reload at every
    Ln/Exp alternation. Stripping the shared funcs from the earlier sets
    (in the cached dict, indices unchanged) routes all of them to
    natural_log_exp_and_others -> a single load for the whole kernel."""
    from concourse.hw_specs import get_activation_tables
    tabs = get_activation_tables(arch)
    target = "natural_log_exp_and_others"
    keep = tabs[target]
    for name, s in tabs.items():
        if name == target:
            break
        s -= keep


def build_program():
    nc = bacc.Bacc("TRN2", target_bir_lowering=False, debug=False,
                   num_devices=N_CORES)
    _pin_act_set(nc.m.arch)

    xT = nc.dram_tensor("xT", [C, T], F16, kind="ExternalInput").ap()
    wT = nc.dram_tensor("wT", [C, 6 * DH], F16, kind="ExternalInput").ap()
    w_oT = nc.dram_tensor("w_oT", [2 * DH, C], F16, kind="ExternalInput").ap()
    cs2 = nc.dram_tensor("cs2", [T, 8 * ROT], F16, kind="ExternalInput").ap()
    nw = nc.dram_tensor("nw", [128, 256], F16, kind="ExternalInput").ap()
    # masks in scoresT layout [key j (partition), query i (free)]
    mf = nc.dram_tensor("mf", [128, 128], F16, kind="ExternalInput").ap()
    ml = nc.dram_tensor("ml", [128, 128], F16, kind="ExternalInput").ap()
    outp = nc.dram_tensor("outp", [T, C], F16, kind="ExternalOutput").ap()

    with tile.TileContext(nc) as tc:
        _body(tc, xT, wT, w_oT, cs2, nw, mf, ml, outp)
    nc.compile()
    return nc


def _body(tc, xT, wT, w_oT, cs2, nw, mf, ml, outp):
    nc = tc.nc
    with ExitStack() as octx:
        singles = octx.enter_context(tc.tile_pool(name="singles", bufs=1))

        # persistent SBUF residents, d-major: head h occupies partitions
        # [64h, 64h+64); score matmuls contract over just that head's 64
        # partitions, so no zero-padding or per-head slots are needed.
        qTr = singles.tile([128, T], F16)
        kTr = singles.tile([128, T], F16)
        # v + ones column, token-major: [tok%128, block, head, 64+1]
        v_all = singles.tile([128, NB, HPC, DH + 1], F16)
        wT_sb = singles.tile([128, 8, 6 * DH], F16)
        w_oT_sb = singles.tile([128, C], F16)
        nw_sb = singles.tile([128, 256], F16)
        mf_sb = singles.tile([128, 128], F16)
        ml_sb = singles.tile([128, 128], F16)
        ident = singles.tile([128, 128], F16)
        eps_sb = singles.tile([128, 1], F32)

        nc.sync.dma_start(wT_sb, wT.rearrange("(a p) n -> p a n", p=128))
        nc.sync.dma_start(w_oT_sb, w_oT)
        nc.sync.dma_start(nw_sb, nw)
        nc.sync.dma_start(mf_sb, mf)
        nc.sync.dma_start(ml_sb, ml)
        make_identity(nc, ident)
        nc.vector.memset(eps_sb, EPS)
        nc.vector.memset(v_all[:, :, :, DH:DH + 1], 1.0)

        xT_r = xT.rearrange("(a p) t -> p a t", p=128)

        # PSUM budget (8 banks, tiles are bank-rounded): pq 2, sc 2, av 1,
        # tp 1, wo 2. The four p@v accumulators of a pair share one av
        # tile; the six transpose slots of an iteration share one tp tile.
        psum_pq = octx.enter_context(
            tc.tile_pool(name="psum_pq", bufs=2, space="PSUM"))
        psum_sc = octx.enter_context(
            tc.tile_pool(name="psum_sc", bufs=2, space="PSUM"))
        psum_tp = octx.enter_context(
            tc.tile_pool(name="psum_tp", bufs=1, space="PSUM"))
        psum_av = octx.enter_context(
            tc.tile_pool(name="psum_av", bufs=1, space="PSUM"))
        psum_wo = octx.enter_context(
            tc.tile_pool(name="psum_wo", bufs=1, space="PSUM"))
        awork = octx.enter_context(tc.tile_pool(name="awork", bufs=5))
        bwork = octx.enter_context(tc.tile_pool(name="bwork", bufs=3))
        bsmall = octx.enter_context(tc.tile_pool(name="bsmall", bufs=4))

        # -------- per-token-block: qkv projection + rmsnorm + rope ---------
        def dma_block(i):
            xt = awork.tile([128, 8, 128], F16, tag="xt", name="xt")
            nc.sync.dma_start(xt, xT_r[:, :, ts(i, 128)])
            csn = awork.tile([128, 2, 4, ROT], F16, tag="csn", name="csn")
            nc.sync.dma_start(csn, cs2[ts(i, 128), :].rearrange(
                "p (t g r) -> p t g r", t=2, g=4))
            return xt, csn[:, 0], csn[:, 1]

        def do_block(i, tiles):
            xt, cs, sn = tiles
            pq = psum_pq.tile([128, 6 * DH], F32, tag="proj", name="pq")
            for j in range(8):
                nc.tensor.matmul(pq, lhsT=xt[:, j, :], rhs=wT_sb[:, j, :],
                                 start=(j == 0), stop=(j == 7))

            # rmsnorm over each of the 4 q/k head groups (64 ch each).
            # HW allows only one non-scalar PSUM operand per instruction, so
            # stage q/k into SBUF fp16 first (also unlocks DVE 2-byte modes).
            qk_sb = awork.tile([128, 256], F16, tag="qk_sb", name="qk_sb")
            nc.vector.tensor_copy(qk_sb, pq[:, 0:256])
            pq4 = qk_sb.rearrange("p (g d) -> p g d", g=4)
            sq = awork.tile([128, 4, DH], F16, tag="sq", name="sq")
            nc.vector.tensor_mul(sq, pq4, pq4)
            ms = awork.tile([128, 4], F32, tag="ms", name="ms")
            nc.vector.reduce_sum(ms, sq, axis=mybir.AxisListType.X)
            # rinv = (ms/DH + eps)^-0.5 via Ln+Exp: both live in the same
            # activation-function set as the attention Exp, so interleaving
            # them never triggers a 1.3us activation-table reload.
            lg = awork.tile([128, 4], F32, tag="lg", name="lg")
            nc.scalar.activation(lg, ms, mybir.ActivationFunctionType.Ln,
                                 bias=eps_sb, scale=1.0 / DH)
            rinv = awork.tile([128, 4], F32, tag="rinv", name="rinv")
            nc.scalar.activation(rinv, lg, mybir.ActivationFunctionType.Exp,
                                 scale=-0.5)

            qkn = awork.tile([128, 256], F16, tag="qkn", name="qkn")
            qkn4 = qkn.rearrange("p (g d) -> p g d", g=4)
            nw4 = nw_sb.rearrange("p (g d) -> p g d", g=4)
            for g in range(4):
                nc.vector.scalar_tensor_tensor(
                    qkn4[:, g, :], pq4[:, g, :], rinv[:, g:g + 1],
                    nw4[:, g, :], mybir.AluOpType.mult,
                    mybir.AluOpType.mult)

            # half-rope on first 32 channels of each group:
            # q' = q*C + rot(q)*S, rot(q) = interleave(-odd, even)
            r4v = qkn4[:, :, 0:ROT].rearrange("p g (a two) -> p g a two",
                                              two=2)
            even = r4v[:, :, :, 0]
            odd = r4v[:, :, :, 1]
            # rot = [odd, even] via one pair-reversed copy; the sign of the
            # odd lane is folded into the host-built sin table (s4 rows are
            # [-s, +s] interleaved), so no separate negate op is needed.
            rot = awork.tile([128, 4, ROT], F16, tag="rot", name="rot")
            rot2 = rot.rearrange("p g (a two) -> p g a two", two=2)
            nc.gpsimd.tensor_copy(rot2, r4v[:, :, :, ::-1])
            nc.gpsimd.tensor_mul(rot, rot, sn)
            f16v = qkn4[:, :, 0:ROT]
            nc.gpsimd.tensor_mul(f16v, f16v, cs)
            nc.gpsimd.tensor_add(f16v, f16v, rot)

            # GPSIMD cannot read PSUM on HW; the Act engine stages v out
            nc.scalar.copy(
                v_all[:, i, :, 0:DH],
                pq[:, 256:384].rearrange("p (h d) -> p h d", h=HPC))
            return qkn

        # transposes are emitted separately, AFTER attention matmuls, so
        # the PE queue never head-of-line blocks on the pool rope chain
        def do_block_tp(i, qkn, tp6, sl):
            tpq = tp6[:, sl, :]
            nc.tensor.transpose(tpq, qkn[:, 0:128], ident)
            nc.vector.tensor_copy(qTr[:, ts(i, 128)], tpq)
            tpk = tp6[:, sl + 1, :]
            nc.tensor.transpose(tpk, qkn[:, 128:256], ident)
            nc.vector.tensor_copy(kTr[:, ts(i, 128)], tpk)

        # -------- per 256-query pair: attention + output projection --------
        attn_tiles = {}

        def emit_unit(p, h, av4):
            nkp = min(2 * p, LOOKBACK) + 2
            k0 = 2 * p + 1 - (nkp - 1)
            hp = 64 * h
            pT = bwork.tile([128, nkp, 256], F16, tag="pT", name="pT")
            # scoresT chunks (2 per PSUM bank) + exp. The masked chunk
            # pairs are computed FIRST so their pool-engine mask multiplies
            # overlap the remaining exps instead of delaying p@v.
            pairs = [(j, min(2, nkp - j)) for j in range(0, nkp, 2)]
            pairs = pairs[-1:] + pairs[:-1]
            for j, w in pairs:
                sc = psum_sc.tile([128, w, 256], F32, tag="sc", name="sc")
                for jj in range(w):
                    nc.tensor.matmul(
                        sc[:, jj, :],
                        lhsT=kTr[64 * h:64 * h + 64, ts(k0 + j + jj, 128)],
                        rhs=qTr[64 * h:64 * h + 64, ts(p, 256)],
                        start=True, stop=True)
                nc.scalar.activation(pT[:, j:j + w, :], sc,
                                     mybir.ActivationFunctionType.Exp,
                                     scale=SCALE)
                # window-edge + causal masks (multiplicative, post-exp):
                # the two masked quarters of a chunk pair form a stride-3
                # diagonal of its quarter view, masked in ONE op against a
                # broadcast mask tile
                if w == 2 and j == 0 and p >= 4:
                    quad = pT[:, 0:2, :].rearrange(
                        "p c (b n) -> p (c b) n", n=128)
                    nc.gpsimd.tensor_mul(
                        quad[:, 0::3, :], quad[:, 0::3, :],
                        mf_sb.rearrange("p (c n) -> p c n", c=1).broadcast_to([128, 2, 128]))
                if w == 2 and j == nkp - 2:
                    quad = pT[:, nkp - 2:nkp, :].rearrange(
                        "p c (b n) -> p (c b) n", n=128)
                    nc.gpsimd.tensor_mul(
                        quad[:, 0::3, :], quad[:, 0::3, :],
                        ml_sb.rearrange("p (c n) -> p c n", c=1).broadcast_to([128, 2, 128]))
            # p @ [v | 1] per 128-query half
            for q in (0, 1):
                blk = 2 * p + q
                lo = 1 if (q == 1 and p >= 4) else 0
                hi = nkp - 2 if q == 0 else nkp - 1
                av = av4[:, 2 * h + q, :]
                js = list(range(lo, hi + 1))
                for m, jx in enumerate(js):
                    nc.tensor.matmul(
                        av, lhsT=pT[:, jx, 128 * q:128 * q + 128],
                        rhs=v_all[:, k0 + jx, h, :],
                        start=(m == 0), stop=(m == len(js) - 1))

        def emit_tail(blk, tp):
            at = attn_tiles.pop(blk)
            nc.tensor.transpose(tp, at, ident)
            aT = bsmall.tile([128, 128], F16, tag="aT", name="aT")
            nc.vector.tensor_copy(aT, tp)
            wo_ps = psum_wo.tile([128, C], F32, tag="wo", name="wo_ps")
            nc.tensor.matmul(wo_ps[:, 0:512], lhsT=aT,
                             rhs=w_oT_sb[:, 0:512], start=True, stop=True)
            nc.tensor.matmul(wo_ps[:, 512:1024], lhsT=aT,
                             rhs=w_oT_sb[:, 512:1024], start=True,
                             stop=True)
            wo_sb = bwork.tile([128, C], F16, tag="wo_sb", name="wo_sb")
            nc.vector.tensor_copy(wo_sb, wo_ps)
            nc.sync.dma_start(outp[ts(blk, 128), :], wo_sb)

        # -------- interleaved schedule: qkv for pair p, then attention -----
        # Software-pipelined, attention-first: each iteration leads with the
        # attention of pair p (inputs ready since last iteration), then the
        # qkv/rope for pair p+1 fills the engines' tails. x/cos/sin DMAs are
        # issued a full iteration ahead of their compute.
        pending = []
        tp0 = psum_tp.tile([128, 6, 128], F16, tag="tp", name="tp0")
        tp1 = psum_tp.tile([128, 6, 128], F16, tag="tp", name="tp1")
        t0, t1 = dma_block(0), dma_block(1)
        t2, t3 = dma_block(2), dma_block(3)
        do_block_tp(0, do_block(0, t0), tp0, 0)
        do_block_tp(1, do_block(1, t1), tp0, 2)
        do_block_tp(2, do_block(2, t2), tp1, 0)
        do_block_tp(3, do_block(3, t3), tp1, 2)
        nxt = (dma_block(4), dma_block(5)) if NP > 2 else None
        for p in range(NP):
            tp6 = psum_tp.tile([128, 6, 128], F16, tag="tp", name="tp6")
            if p + 3 < NP:
                cur, nxt = nxt, (dma_block(2 * p + 6), dma_block(2 * p + 7))
            else:
                cur, nxt = nxt, None
            attn_tiles[2 * p] = bsmall.tile([128, 128], F16,
                                            tag="attn", name="attn_a")
            attn_tiles[2 * p + 1] = bsmall.tile([128, 128], F16,
                                                tag="attn", name="attn_b")
            av4 = psum_av.tile([128, 4, DH + 1], F32, tag="av", name="av4")
            emit_unit(p, 0, av4)
            if cur is not None:
                qa = do_block(2 * p + 4, cur[0])
            for n, blk in enumerate(pending):
                emit_tail(blk, tp6[:, 4 + n, :])
            pending = []
            emit_unit(p, 1, av4)
            if cur is not None:
                qb = do_block(2 * p + 5, cur[1])
                do_block_tp(2 * p + 4, qa, tp6, 0)
                do_block_tp(2 * p + 5, qb, tp6, 2)
            r4 = bsmall.tile([128, 4], F32, tag="r4", name="r4")
            r4s = bsmall.tile([128, 4], F32, tag="r4s", name="r4s")
            nc.vector.reciprocal_approx_accurate(r4, av4[:, :, DH], r4s)
            # normalize straight out of PSUM into the attn tiles: one TSP
            # replaces the copy + in-place scale
            for h in range(2):
                for q in range(2):
                    nc.vector.tensor_scalar_mul(
                        attn_tiles[2 * p + q][:, 64 * h:64 * h + 64],
                        av4[:, 2 * h + q, 0:DH],
                        r4[:, 2 * h + q:2 * h + q + 1])
            pending = [2 * p, 2 * p + 1]
        tp_last = psum_tp.tile([128, 6, 128], F16, tag="tp", name="tp_last")
        for n, blk in enumerate(pending):
            emit_tail(blk, tp_last[:, n, :])


_PROGRAM = None


def _get_program():
    global _PROGRAM
    if _PROGRAM is None:
        _PROGRAM = build_program()
    return _PROGRAM


def make_in_maps(x, w_qkv, w_o, cos, sin, pos):
    """Host-side sharding: build the per-core input dicts."""
    xT = np.ascontiguousarray(
        np.asarray(x, np.float32).reshape(T, C).T).astype(np.float16)

    cos_u = np.asarray(cos, np.float32)[np.asarray(pos).reshape(-1)]
    sin_u = np.asarray(sin, np.float32)[np.asarray(pos).reshape(-1)]
    cdup = np.empty((T, ROT), np.float32)
    cdup[:, 0::2] = cos_u
    cdup[:, 1::2] = cos_u
    sdup = np.empty((T, ROT), np.float32)
    sdup[:, 0::2] = -sin_u
    sdup[:, 1::2] = sin_u
    cs2 = np.concatenate([np.tile(cdup, (1, 4)), np.tile(sdup, (1, 4))],
                         axis=1).astype(np.float16)

    ones = np.ones((128, 128), np.float16)
    mf = np.ascontiguousarray(np.tril(ones, -1))  # allowed iff q_i < key_j
    ml = np.ascontiguousarray(np.triu(ones, 0))   # allowed iff q_i >= key_j

    w_qkv = np.asarray(w_qkv, np.float32)
    w_o = np.asarray(w_o, np.float32)
    in_maps = []
    for c in range(N_CORES):
        h0, h1 = HPC * c, HPC * c + 1
        rows = np.r_[h0 * DH:(h0 + 1) * DH, h1 * DH:(h1 + 1) * DH]
        w_shard = np.concatenate(
            [w_qkv[rows], w_qkv[C + rows], w_qkv[2 * C + rows]], axis=0)
        wT_c = np.ascontiguousarray(w_shard.T).astype(np.float16)
        w_oT_c = np.ascontiguousarray((0.5 * w_o[:, rows]).T).astype(np.float16)
        in_maps.append({
            "xT": xT, "wT": wT_c, "w_oT": w_oT_c,
            "cs2": cs2, "nw": None, "mf": mf, "ml": ml,
        })
    return in_maps


def _norm_weight_tile(q_norm_w, k_norm_w):
    nw = np.concatenate([np.tile(np.asarray(q_norm_w, np.float32), HPC),
                         np.tile(np.asarray(k_norm_w, np.float32), HPC)])
    return np.ascontiguousarray(np.broadcast_to(nw, (128, 256))).astype(
        np.float16)


def kernel(x, tokens, pos, w_qkv, w_o, q_norm_w, k_norm_w, cos, sin,
           window_tokens, block_size):
    assert int(window_tokens) == WINDOW and int(block_size) == 128
    nc = _get_program()
    in_maps = make_in_maps(x, w_qkv, w_o, cos, sin, pos)
    nw_t = _norm_weight_tile(q_norm_w, k_norm_w)
    for m in in_maps:
        m["nw"] = nw_t

    res = run_bass_kernel_spmd(nc, in_maps, list(range(N_CORES)))
    out = np.zeros((T, C), np.float64)
    for c in range(N_CORES):
        out += res.results[c]["outp"].astype(np.float64)
    return out.astype(np.float32).reshape(1, T, C)

